# revision 64
# baseline (speedup 1.0000x reference)
"""Sliding-window multi-head attention (N=4, T=2048, D=1024, H=16, hd=64,
full-dim rotary, window (128,128)) on 8 Trainium2 NeuronCores.

Sharding: (batch, head-half): core c handles batch c//2 and heads
[8*(c%2), 8*(c%2)+8) over the FULL sequence — no halo recompute. Each core
emits a partial out-projection (contraction over its 512 features); the host
sums the two partials per batch and adds bout.

Per-core program (all matmuls bf16, fp32 PSUM accumulate):
  P1  qkv projection from feature-major x; RoPE on q,k via a PE
      half-swap permutation matmul + cos/signed-sin tables.
      V token-major with per-head [64 v | 64 ones] column blocks.
  P3  banded attention per (head, 512-query block): score stripes
      [128 keys x <=384 queries] packed into one [128,1536] PSUM tile
      (bank-aligned), single exp, single band-mask multiply, ordered-start
      AV accumulation -> psA = [64 attnout rows | 64 replicated sum rows],
      one divide normalizes.
  P4  partial out-projection interleaved per query block; bf16 output.
"""

import math

import ml_dtypes
import numpy as np

import bass_rust
import concourse.bass as bass
import concourse.mybir as mybir
import concourse.tile as tile
from concourse.bass_utils import run_bass_kernel_spmd
from concourse.vector_clock import ScopedClock

# ----------------------------------------------------------------------------
N, T, D = 4, 2048, 1024
H, HD = 16, 64
HLOC = 8            # heads per core
FH = HLOC * HD      # 512 q/k/v features per core
WIN = 128
ROPE_BASE = 10000.0
SCALE = 1.0 / math.sqrt(HD)

NCORES = 8
QB = 512
NQB = T // QB       # 4
NKT = T // 128      # 16
SW = 1536           # packed score-tile width (3 PSUM banks)

F32 = mybir.dt.float32
BF16 = mybir.dt.bfloat16

# debug: add DRAM taps for intermediates (dev only; off for grading)
DEBUG_TAPS = False

_MAXW = 1  # this container's walrus accepts one sync wait per instruction


class SplitWaitTC(tile.TileContext):
    """TileContext that spreads multi-sem waits over NoOp carriers so every
    instruction carries at most one sync wait (codegen limit here)."""

    _waitnop_counter = 0

    def _split_waits(self, inst, commit):
        si = getattr(inst, "sync_info", None)
        if si is None:
            return
        waits = list(si.on_wait)
        if len(waits) <= _MAXW:
            return
        ups = list(si.on_update)
        head, keep = waits[:-_MAXW], waits[-_MAXW:]
        for w in head:
            nop = bass_rust.InstNoOp()
            nop.engine = inst.engine
            SplitWaitTC._waitnop_counter += 1
            nop.name = f"I-waitnop-{SplitWaitTC._waitnop_counter}"
            nop.bass_nofuse = True
            nop.sync_info = bass_rust.SyncInfo(on_wait=[w], on_update=[])
            commit(nop)
        inst.sync_info = bass_rust.SyncInfo(on_wait=keep, on_update=ups)

    def _commit_and_lower(self, inst, original_block, old_bb_map, bb_to_exit_bb):
        if isinstance(inst, mybir.Instruction) and not isinstance(
            inst, (tile.BassTileRelease,)
        ):
            self._split_waits(
                inst,
                lambda nop: super(SplitWaitTC, self)._commit_and_lower(
                    nop, original_block, old_bb_map, bb_to_exit_bb
                ),
            )
        return super()._commit_and_lower(inst, original_block, old_bb_map, bb_to_exit_bb)

    def _drain_and_barrier(self, tick_clock, wait_clock):
        probe = self.nc.sync.nop(nofuse=True)
        wait_clock.add_sem_waits(probe.ins, ScopedClock({None: tick_clock.global_clock}))
        si = probe.ins.sync_info
        waits = list(si.on_wait) if si is not None else []
        ups = list(si.on_update) if si is not None else []
        if len(waits) > _MAXW:
            probe.ins.sync_info = bass_rust.SyncInfo(on_wait=waits[:_MAXW], on_update=ups)
            rest = waits[_MAXW:]
            while rest:
                chunk, rest = rest[:_MAXW], rest[_MAXW:]
                n = self.nc.sync.nop(nofuse=True)
                n.ins.sync_info = bass_rust.SyncInfo(on_wait=chunk, on_update=[])
        self.nc.sync.drain()
        self.nc.all_engine_barrier()
        assert self.sems is not None
        popped = self.nc._tile_sem_poison_stack.pop()
        assert popped is self._sem_poison
        self.nc.clear_and_free_semaphores(list(self.sems.allocated().values()))
        self.nc.all_engine_barrier()


# ----------------------------------------------------------------------------
# Static stripe planning (shared by device codegen and host mask builder)


def qb_stripes(qb):
    """Score stripes for query block qb: list of (kt, off, end) with
    queries [qb*QB+off, qb*QB+end) valid for key tile kt."""
    q0 = qb * QB
    res = []
    for kt in range(max(0, q0 // 128 - 1), min(NKT, q0 // 128 + 5)):
        off = max(0, 128 * (kt - 1) - q0)
        end = min(QB, 128 * (kt + 2) - q0)
        if end > off:
            res.append((kt, off, end))
    return res


def pack_stripes(stripes):
    """Place stripes in a [128, SW] tile without crossing 512-col PSUM bank
    boundaries. Returns (placed=[(kt,off,end,col)], holes=[(c0,c1)])."""
    banks = [[] for _ in range(SW // 512)]
    fill = [0] * (SW // 512)
    for kt, off, end in sorted(stripes, key=lambda s: -(s[2] - s[1])):
        w = end - off
        for b in range(len(banks)):
            if fill[b] + w <= 512:
                banks[b].append((kt, off, end, b * 512 + fill[b]))
                fill[b] += w
                break
        else:
            raise AssertionError("stripe packing overflow")
    placed = [s for b in banks for s in b]
    holes = [
        (b * 512 + fill[b], (b + 1) * 512)
        for b in range(len(banks))
        if fill[b] < 512
    ]
    return placed, holes


def av_plan(placed):
    """AV matmul schedule. PSUM semantics: start=True clears the target
    bank's has_written bits and writes its range; start=False writes
    unwritten elements and accumulates written ones. So: first matmul
    start=True, all others start=False — per-element first-writer handling
    is automatic. Returns [(kt, joff, jend, pcol, start)]."""
    cov = np.zeros(QB, bool)
    for _, off, end, _ in placed:
        cov[off:end] = True
    assert cov.all()
    return [
        (kt, off, end, col, i == 0)
        for i, (kt, off, end, col) in enumerate(placed)
    ]


QB_PLANS = []
for _qb in range(NQB):
    _placed, _holes = pack_stripes(qb_stripes(_qb))
    QB_PLANS.append((_placed, _holes, av_plan(_placed)))
MASK_VARIANT = {0: 0, 1: 1, 2: 1, 3: 2}


# ----------------------------------------------------------------------------
# Device program


def build_nc():
    nc = bass.Bass("TRN2", target_bir_lowering=False, debug=False, num_devices=NCORES)

    xt = nc.declare_dram_parameter("xt", [D, T], BF16, isOutput=False)
    wq = nc.declare_dram_parameter("wq", [D, FH], BF16, isOutput=False)
    wk = nc.declare_dram_parameter("wk", [D, FH], BF16, isOutput=False)
    wv = nc.declare_dram_parameter("wv", [D, FH], BF16, isOutput=False)
    wo = nc.declare_dram_parameter("wo", [FH, D], BF16, isOutput=False)
    ctabd = nc.declare_dram_parameter("ctab", [128, T], BF16, isOutput=False)
    stabd = nc.declare_dram_parameter("stab", [128, T], BF16, isOutput=False)
    maskd = nc.declare_dram_parameter("mask", [128, 3 * SW], BF16, isOutput=False)
    permd = nc.declare_dram_parameter("perm", [128, 128], BF16, isOutput=False)
    yt = nc.declare_dram_parameter("yt", [D, T], BF16, isOutput=True)
    dbg = {}
    if DEBUG_TAPS:
        for nm, shp in [
            ("dbg_qT0", [128, T]), ("dbg_kT0", [128, T]), ("dbg_vp0", [128, 1024]),
            ("dbg_probs", [128, SW]), ("dbg_aT0", [128, T]),
            ("dbg_psA", [128, QB]),
        ]:
            dbg[nm] = nc.declare_dram_parameter(nm, shp, BF16, isOutput=True)

    AF = mybir.ActivationFunctionType
    ALU = mybir.AluOpType

    with nc.allow_low_precision(reason="bf16 matmul inputs; fp32 accumulate"), \
            SplitWaitTC(nc) as tc:
        with (
            tc.tile_pool(name="const", bufs=1) as constp,
            tc.tile_pool(name="persist", bufs=1) as persist,
        ):
            perm_t = constp.tile([128, 128], BF16, name="perm", tag="perm")
            zbf_t = constp.tile([128, 128], BF16, name="zbf", tag="zbf")
            nc.vector.memset(zbf_t[:], 0.0)
            ctab_t = constp.tile([128, T], BF16, name="ctab", tag="ctab")
            stab_t = constp.tile([128, T], BF16, name="stab", tag="stab")
            mask_t = constp.tile([128, 3 * SW], BF16, name="mask", tag="mask")
            wo_t = constp.tile([128, 4 * D], BF16, name="wo", tag="wo")

            qT = [persist.tile([128, T], BF16, name=f"qT{i}", tag=f"qT{i}") for i in range(4)]
            kT = [persist.tile([128, T], BF16, name=f"kT{i}", tag=f"kT{i}") for i in range(4)]
            vpb = persist.tile([128, NKT * 1024], BF16, name="vpb", tag="vpb")
            vp = [vpb[:, i * 1024:(i + 1) * 1024] for i in range(NKT)]
            aT = [persist.tile([128, T], BF16, name=f"aT{i}", tag=f"aT{i}") for i in range(4)]

            # ---------------- P1: qkv projection + rope ----------------------
            with (
                tc.tile_pool(name="xtp", bufs=1) as xtp,
                tc.tile_pool(name="wp", bufs=1) as wp,
            ):
              xts = [xtp.tile([128, 8 * QB], BF16, name=f"xt{c}", tag=f"xt{c}") for c in range(4)]
              wq_t = wp.tile([128, 8 * FH], BF16, name="wq", tag="wq")
              wk_t = wp.tile([128, 8 * FH], BF16, name="wk", tag="wk")
              wv_t = wp.tile([128, 8 * FH], BF16, name="wv", tag="wv")
              with (
                tc.tile_pool(name="rawp", bufs=4) as rawp,
                tc.tile_pool(name="ropet", bufs=4) as ropet,
                tc.tile_pool(name="psq", bufs=2, space="PSUM") as psqp,
                tc.tile_pool(name="psw", bufs=2, space="PSUM") as pswp,
              ):

                # load order = first-use order: x quarter 0 + Wq unblock the
                # first projection chunk; tables/mask/wo follow the weights
                # first-chunk deps interleaved in consumption order: x
                # quarter 0 d-tile half, Wq half, second halves
                def dma_x0_q(ah):
                    nc.sync.dma_start(
                        xts[0][:].rearrange("p (a t) -> p a t", t=QB)[
                            :, ah * 2:(ah + 1) * 2, :
                        ],
                        xt[:].rearrange("(a p) t -> p a t", p=128)[
                            :, ah * 2:(ah + 1) * 2, :QB
                        ],
                    )

                def dma_wq_half(hf):
                    nc.sync.dma_start(
                        wq_t[:].rearrange("p (a f) -> p a f", f=FH)[
                            :, :, hf * 256:(hf + 1) * 256
                        ],
                        wq[:].rearrange("(a p) f -> p a f", p=128)[
                            :, :, hf * 256:(hf + 1) * 256
                        ],
                    )

                dma_x0_q(0)
                dma_wq_half(0)
                dma_x0_q(1)
                dma_x0_q(2)
                dma_wq_half(1)
                dma_x0_q(3)
                nc.sync.dma_start(perm_t[:], permd[:])
                nc.sync.dma_start(ctab_t[:], ctabd[:])
                nc.sync.dma_start(stab_t[:], stabd[:])
                for c in range(1, 4):
                    nc.sync.dma_start(
                        xts[c][:].rearrange("p (a t) -> p a t", t=QB),
                        xt[:].rearrange("(a p) t -> p a t", p=128)[:, :, c * QB:(c + 1) * QB],
                    )
                nc.sync.dma_start(
                    wv_t[:].rearrange("p (a f) -> p a f", f=FH),
                    wv[:].rearrange("(a p) f -> p a f", p=128),
                )
                nc.sync.dma_start(
                    wk_t[:].rearrange("p (a f) -> p a f", f=FH),
                    wk[:].rearrange("(a p) f -> p a f", p=128),
                )
                # constants needed later in the pipeline
                nc.sync.dma_start(mask_t[:], maskd[:])
                nc.sync.dma_start(
                    wo_t[:].rearrange("p (a f) -> p a f", f=D),
                    wo[:].rearrange("(a p) f -> p a f", p=128),
                )

                # q/k feature tiles with rope; perm-matmuls staggered one
                # chunk behind the projection matmuls to keep PE fed.
                pending = []

                def emit_tail(args):
                    is_q, f, c, psq, raw = args
                    dest = qT[f] if is_q else kT[f]
                    c0 = c * QB
                    psw = pswp.tile([128, QB], F32, name="psw", tag="psw")
                    nc.tensor.matmul(psw[:], perm_t[:], raw[:], start=True, stop=True)
                    t1 = ropet.tile([128, QB], BF16, name="t1", tag="t1")
                    nc.gpsimd.tensor_mul(t1[:], raw[:], ctab_t[:, c0:c0 + QB])
                    t2 = ropet.tile([128, QB], BF16, name="t2", tag="t2")
                    nc.vector.tensor_mul(t2[:], psw[:], stab_t[:, c0:c0 + QB])
                    nc.vector.tensor_add(dest[:, c0:c0 + QB], t1[:], t2[:])

                # token-quarter outer: chunk (f, c) only needs x quarter c,
                # so PE work per arriving DMA quarter is 4 chunks (~7us)
                for is_q, c, f in (
                    (iq, c, f)
                    for iq in (True, False)
                    for c in range(4)
                    for f in range(4)
                ):
                    if True:
                        w_t = wq_t if is_q else wk_t
                        psq = psqp.tile([128, QB], F32, name="psq", tag="psq")
                        for kt8 in range(8):
                            nc.tensor.matmul(
                                psq[:],
                                w_t[:, kt8 * FH + f * 128: kt8 * FH + (f + 1) * 128],
                                xts[c][:, kt8 * QB:(kt8 + 1) * QB],
                                start=(kt8 == 0),
                                stop=(kt8 == 7),
                            )
                        raw = rawp.tile([128, QB], BF16, name="raw", tag="raw")
                        nc.scalar.copy(raw[:], psq[:])
                        if pending:
                            emit_tail(pending.pop())
                        pending.append((is_q, f, c, psq, raw))
                while pending:
                    emit_tail(pending.pop())

            # ---------------- P3 + P4 (+ dripped V build) --------------------
              with (
                tc.tile_pool(name="probs", bufs=7) as probsp,
                tc.tile_pool(name="ysp", bufs=2) as ysp,
                tc.tile_pool(name="srecp", bufs=8) as srecp,
                tc.tile_pool(name="ssbp", bufs=4) as ssbp,
                tc.tile_pool(name="sps", bufs=2, space="PSUM") as sps,
                tc.tile_pool(name="smallps", bufs=2, space="PSUM") as smallps,
              ):
                def emit_v(kt):
                    # V token-major. Per head pair (256 cols):
                    # [v_even 64 | ones 128 | v_odd 64] so even heads' AV
                    # puts attnout on lanes 0-63 (sums on 64-127) and odd
                    # heads the mirror — normalize then writes aT in place.
                    qtr, toff = kt // 4, (kt % 4) * 128
                    pview = vp[kt].rearrange("p (pair x) -> p pair x", x=256)
                    psv = smallps.tile([128, FH], F32, name="psv", tag="small")
                    for kt8 in range(8):
                        nc.tensor.matmul(
                            psv[:],
                            xts[qtr][:, kt8 * QB + toff: kt8 * QB + toff + 128],
                            wv_t[:, kt8 * FH:(kt8 + 1) * FH],
                            start=(kt8 == 0),
                            stop=(kt8 == 7),
                        )
                    # even-head v at pair*256, odd-head v at pair*256+192
                    psvv = psv[:].rearrange("p (pair x) -> p pair x", x=128)
                    nc.scalar.copy(pview[:, :, :HD], psvv[:, :, :HD])
                    nc.scalar.copy(pview[:, :, 192:256], psvv[:, :, HD:])

                def attn_unit_front(h, qb):
                    """QK stripes + exp + mask, pipelined per PSUM bank so
                    the probs for bank b are ready ~1 bank after its QKs."""
                    placed, holes, _ = QB_PLANS[qb]
                    v = MASK_VARIANT[qb]
                    f, p0 = h // 2, (h % 2) * HD
                    q0 = qb * QB
                    psS = sps.tile([128, SW], F32, name="psS", tag="psS")
                    probs = probsp.tile([128, SW], BF16, name="probs", tag="probs")
                    for kt, off, end, col in placed:
                        nc.tensor.matmul(
                            psS[:, col:col + end - off],
                            kT[f][p0:p0 + HD, kt * 128:(kt + 1) * 128],
                            qT[f][p0:p0 + HD, q0 + off:q0 + end],
                            start=True,
                            stop=True,
                            skip_group_check=True,
                        )
                    for c0, c1 in holes:
                        nc.tensor.matmul(
                            psS[:, c0:c1],
                            zbf_t[:],
                            zbf_t[:, : c1 - c0],
                            start=True,
                            stop=True,
                            skip_group_check=True,
                        )
                    nc.scalar.activation(probs[:], psS[:], AF.Exp, scale=SCALE)
                    # band mask split: first bank on DVE (feeds the first AV
                    # matmuls fastest), rest on the gpsimd engine
                    nc.vector.tensor_mul(
                        probs[:, :512], probs[:, :512], mask_t[:, v * SW:v * SW + 512]
                    )
                    nc.gpsimd.tensor_mul(
                        probs[:, 512:], probs[:, 512:], mask_t[:, v * SW + 512:(v + 1) * SW]
                    )
                    if DEBUG_TAPS and h == 0 and qb == 0:
                        nc.sync.dma_start(dbg["dbg_probs"][:], probs[:])
                    return probs

                def attn_unit_back(h, qb, probs):
                    """AV accumulation + normalize prep for (head, qb).
                    The final multiply is deferred (returned as a thunk) so
                    its lane-shift DMA never head-of-line blocks an engine."""
                    _, _, mms = QB_PLANS[qb]
                    f, p1 = h // 2, (h % 2) * HD
                    q0 = qb * QB
                    psA = smallps.tile([128, QB], F32, name="psA", tag="small")
                    for i, (kt, j, j2, cs, st) in enumerate(mms):
                        nc.tensor.matmul(
                            psA[:, j:j2],
                            vp[kt][:, h * 128:(h + 1) * 128],
                            probs[:, cs:cs + (j2 - j)],
                            start=st,
                            stop=(i == len(mms) - 1),
                            skip_group_check=True,
                        )
                    # attnout lanes: even head 0-63 (sums 64-127), odd head
                    # 64-127 (sums 0-63). Sums go PSUM->SBUF on their own
                    # lanes, an SBUF->SBUF DMA shifts them onto the attnout
                    # lanes, then one aligned divide writes aT in place.
                    if DEBUG_TAPS and h == 0 and qb == 0:
                        dbt = srecp.tile([128, QB], BF16, name="dbt", tag="dbt")
                        nc.scalar.copy(dbt[:], psA[:])
                        nc.sync.dma_start(dbg["dbg_psA"][:], dbt[:])
                    # Copy attnout to SBUF (ACT) + reciprocal of sums (DVE)
                    # release psA after ~1.5us; the lane-shift DMA and the
                    # all-SBUF bf16 multiply then run off the critical path.
                    dst = aT[f][p1:p1 + HD, q0:q0 + QB]
                    acp = srecp.tile([128, QB], BF16, name="acp", tag="acp")
                    ssb = ssbp.tile([128, QB], BF16, name="ssb", tag="ssb")
                    srec = srecp.tile([128, QB], BF16, name="srec", tag="srec")
                    if h % 2 == 0:
                        nc.scalar.copy(acp[:HD, :], psA[:HD, :])
                        nc.vector.reciprocal(ssb[HD:128, :], psA[HD:128, :])
                        nc.sync.dma_start(srec[:HD, :], ssb[HD:128, :])
                        return lambda: nc.vector.tensor_mul(
                            dst, acp[:HD, :], srec[:HD, :]
                        )
                    else:
                        nc.vector.tensor_copy(acp[HD:128, :], psA[HD:128, :])
                        nc.vector.reciprocal(ssb[:HD, :], psA[:HD, :])
                        nc.sync.dma_start(srec[HD:128, :], ssb[:HD, :])
                        return lambda: nc.vector.tensor_mul(
                            dst, acp[HD:128, :], srec[HD:128, :]
                        )

                def p4_piece(qb, part):
                    """Out-projection piece: 2 output feature tiles of qb."""
                    ys = ysp.tile([128, 2 * QB], BF16, name="ys", tag="ys")
                    for mi in range(2):
                        mo = part * 2 + mi
                        psy = smallps.tile([128, QB], F32, name="psy", tag="small")
                        for kf in range(4):
                            nc.tensor.matmul(
                                psy[:],
                                wo_t[:, kf * D + mo * 128: kf * D + (mo + 1) * 128],
                                aT[kf][:, qb * QB:(qb + 1) * QB],
                                start=(kf == 0),
                                stop=(kf == 3),
                            )
                        cp = nc.scalar.copy if mi == 0 else nc.vector.tensor_copy
                        cp(ys[:, mi * QB:(mi + 1) * QB], psy[:])
                    nc.sync.dma_start(
                        yt[:].rearrange("(a pp) t -> pp a t", pp=128)[
                            :, part * 2:(part + 1) * 2, qb * QB:(qb + 1) * QB
                        ],
                        ys[:].rearrange("p (a t) -> p a t", t=QB),
                    )

                # Stream attention units with the AV two units behind the
                # QK+exp front (PE lookahead > exp+mask latency). Deferred
                # work (normalize muls, then 2-mo P4 pieces) from the
                # previous query block drips out one item per unit so it
                # never bursts against the next block's exp/mask chain.
                backlog = []
                muls = {qb: [] for qb in range(NQB)}
                dripq = []
                nc.vector.memset(
                    vpb[:].rearrange("p (q x) -> p q x", x=256)[:, :, HD:HD + 128], 1.0
                )
                for _kt in range(5):  # vp[0..4] cover query block 0
                    emit_v(_kt)
                vnext = [5]

                def drain_unit(bh, bqb, bp):
                    muls[bqb].append(attn_unit_back(bh, bqb, bp))
                    if len(muls[bqb]) == HLOC:
                        # all of bqb's muls known: queue them + P4 pieces
                        dripq.extend(muls[bqb])
                        for part in range(4):
                            dripq.append(lambda q=bqb, p=part: p4_piece(q, p))

                for qb in range(NQB):
                    horder = range(HLOC)
                    for h in horder:
                        probs = attn_unit_front(h, qb)
                        backlog.append((h, qb, probs))
                        if len(backlog) > 4:
                            drain_unit(*backlog.pop(0))
                        if vnext[0] < NKT:
                            emit_v(vnext[0])
                            vnext[0] += 1
                        for _ in range(2 if dripq else 0):
                            if dripq:
                                dripq.pop(0)()
                for u in backlog:
                    drain_unit(*u)
                while dripq:
                    dripq.pop(0)()

                if DEBUG_TAPS:
                    nc.sync.dma_start(dbg["dbg_qT0"][:], qT[0][:])
                    nc.sync.dma_start(dbg["dbg_kT0"][:], kT[0][:])
                    nc.sync.dma_start(dbg["dbg_vp0"][:], vp[0][:])
                    nc.sync.dma_start(dbg["dbg_aT0"][:], aT[0][:])

    return nc


# ----------------------------------------------------------------------------
# Host-side shard preparation


def _rope_tables():
    """[128, T] cos and signed-sin tables for the 2-head tile row layout."""
    inv_freq = 1.0 / (ROPE_BASE ** (np.arange(0, HD, 2, dtype=np.float32) / HD))
    freqs = np.outer(np.arange(T, dtype=np.float32), inv_freq)  # [T, 32]
    c32 = np.cos(freqs).astype(np.float32).T
    s32 = np.sin(freqs).astype(np.float32).T
    ctab = np.tile(c32, (4, 1))
    sgn = np.repeat(np.array([-1.0, 1.0, -1.0, 1.0], dtype=np.float32), 32)
    stab = np.tile(s32, (4, 1)) * sgn[:, None]
    return ctab, stab


def _perm_matrix():
    p = np.zeros((128, 128), dtype=np.float32)
    for i in range(128):
        j = i + 32 if (i // 32) % 2 == 0 else i - 32
        p[i, j] = 1.0
    return p


def _build_masks():
    m = np.zeros((128, 3 * SW), dtype=np.float32)
    for vi, qb in enumerate([0, 1, 3]):
        placed, _, _ = QB_PLANS[qb]
        q0 = qb * QB
        for kt, off, end, col in placed:
            kk = kt * 128 + np.arange(128)[:, None]
            qq = q0 + np.arange(off, end)[None, :]
            m[:, vi * SW + col: vi * SW + col + end - off] = (
                np.abs(kk - qq) <= WIN
            )
    return m


_BF = ml_dtypes.bfloat16


def _core_inputs(x, Wqkv, Wout, core):
    n, hg = core // 2, core % 2
    f0 = hg * FH
    common = _CORE_COMMON
    return {
        "xt": np.ascontiguousarray(x[n].T).astype(_BF),
        "wq": np.ascontiguousarray(Wqkv[:, f0:f0 + FH]).astype(_BF),
        "wk": np.ascontiguousarray(Wqkv[:, D + f0:D + f0 + FH]).astype(_BF),
        "wv": np.ascontiguousarray(Wqkv[:, 2 * D + f0:2 * D + f0 + FH]).astype(_BF),
        "wo": np.ascontiguousarray(Wout[f0:f0 + FH, :]).astype(_BF),
        **common,
    }


_CORE_COMMON = None


def _common_inputs():
    global _CORE_COMMON
    if _CORE_COMMON is None:
        ctab, stab = _rope_tables()
        _CORE_COMMON = {
            "ctab": ctab.astype(_BF),
            "stab": stab.astype(_BF),
            "mask": _build_masks().astype(_BF),
            "perm": _perm_matrix().astype(_BF),
        }
    return _CORE_COMMON


_NC_CACHE = {}


def _get_nc():
    if "nc" not in _NC_CACHE:
        _NC_CACHE["nc"] = build_nc()
    return _NC_CACHE["nc"]


def kernel(x, Wqkv, Wout, bout, _trace=False, _trace_kwargs=None):
    x = np.asarray(x, dtype=np.float32)
    Wqkv = np.asarray(Wqkv, dtype=np.float32)
    Wout = np.asarray(Wout, dtype=np.float32)
    _common_inputs()
    in_maps = [_core_inputs(x, Wqkv, Wout, c) for c in range(NCORES)]
    nc = _get_nc()
    kw = {}
    if _trace:
        kw = {"trace": True, "trace_kwargs": _trace_kwargs or {}}
    res = run_bass_kernel_spmd(nc, in_maps, core_ids=list(range(NCORES)), **kw)
    out = np.zeros((N, T, D), dtype=np.float32)
    for c in range(NCORES):
        n = c // 2
        out[n] += np.asarray(res.results[c]["yt"], dtype=np.float32).T
    out += np.asarray(bout, dtype=np.float32)[None, None, :]
    kernel._last_results = res
    return out
